# revision 44
# baseline (speedup 1.0000x reference)
"""Trainium2 Bass kernel for nn_MultiHeadAttention_8684423872640.

Math: the reference collapses algebraically. With
  s[m]   = Wfc[0, m // 64] / sqrt(64)
  Abar   = (Wk * s[:,None]).T @ Wq / L          # [1024, 1024] weights-only
  u      = Wk.T @ (s * bq)                      # [1024]
  qv     = Wq.T @ (s * bk) / L                  # [1024]
  c0     = (s * bk) @ bq + bfc[0]
the output for batch b is
  xsum_b = sum_l x[b, l, :]                     # [1024]
  w_eff  = Abar @ xsum_b + u                    # [1024]
  c      = qv @ xsum_b + c0
  out[b, l, 0] = x[b, l, :] @ w_eff + c

Sharding: data-parallel over B — core c handles batch c. Each core:
  pass 1: DMA x[b].T tiles [128, 4096] to SBUF, VectorE row-sums -> xsum
          (incrementally per 128-feature tile), TensorE folds each xsum
          p-tile into w_eff/c via Abar-block matmuls as soon as it's ready
  pass 2: TensorE matvec out[l] = xT[:, l] . w_eff (w_eff stationary,
          x streams as moving operand), +c epilogue on VectorE, DMA out.
"""

import os
import sys
import functools
import numpy as np

B, L, N = 8, 4096, 1024
D_K = 64
NCORES = 8
PT = N // 128  # 8 feature tiles
LCH = 512      # pass-2 moving chunk (fp32 max)
NLC = L // LCH

_TRN_REPO = "/opt/trn_rl_repo"


def _ensure_path():
    if _TRN_REPO not in sys.path and os.path.isdir(_TRN_REPO):
        sys.path.insert(0, _TRN_REPO)


@functools.lru_cache(maxsize=2)
def _build(x_dt_name: str = "float32", tail_split: int = 4, warmup_mms: int = 10):
    """Build + compile the per-core Bass program. Returns the finalized nc."""
    _ensure_path()
    import concourse.bass as bass
    import concourse.tile as tile
    from concourse import bacc, mybir

    f32 = mybir.dt.float32
    dtx = getattr(mybir.dt, x_dt_name)

    nc = bacc.Bacc(
        "TRN2",
        target_bir_lowering=False,
        debug=False,
        enable_asserts=False,
        num_devices=NCORES,
    )

    xT = nc.dram_tensor("xT", [N, L], dtx, kind="ExternalInput").ap()
    atr = nc.dram_tensor("atr", [128, PT * N], dtx, kind="ExternalInput").ap()
    qv8 = nc.dram_tensor("qv8", [128, PT], dtx, kind="ExternalInput").ap()
    u8 = nc.dram_tensor("u8", [128, PT], f32, kind="ExternalInput").ap()
    c0 = nc.dram_tensor("c0", [1, 1], f32, kind="ExternalInput").ap()
    out_d = nc.dram_tensor("out", [1, L], f32, kind="ExternalOutput").ap()

    with tile.TileContext(nc) as tc:
        with (
            tc.tile_pool(name="xpool", bufs=PT) as xpool,
            tc.tile_pool(name="cpool", bufs=1) as cpool,
            tc.tile_pool(name="spool", bufs=2) as spool,
            tc.tile_pool(name="xsums", bufs=PT + 2) as xsums,
            tc.tile_pool(name="wps", bufs=2, space="PSUM") as wps,
            tc.tile_pool(name="cps", bufs=1, space="PSUM") as cps,
            tc.tile_pool(name="ops", bufs=3, space="PSUM") as ops,
            tc.tile_pool(name="wrm", bufs=1, space="PSUM") as wrm,
        ):
            # Three DMA queues: the two HWDGE rings (SP + ACT) carry x
            # tiles in alternation (one ring's ~2us completion receipt
            # hides under the other's data); weights ride SWDGE (gpsimd)
            # so they never delay an x tile.
            rings = [nc.sync, nc.scalar]

            # -- small constants up front (SWDGE; tiny) --
            at_sb = cpool.tile([128, PT * N], dtx, tag="at")
            qv_sb = cpool.tile([128, PT], dtx, tag="qv")
            nc.gpsimd.dma_start(qv_sb[:], qv8[:])
            u_sb = cpool.tile([128, PT], f32, tag="u")
            nc.gpsimd.dma_start(u_sb[:], u8[:])
            c0_sb = cpool.tile([1, 1], f32, tag="c0")
            nc.gpsimd.dma_start(c0_sb[:], c0[:])

            # -- x tiles interleaved with the two at halves --
            # Per-ring FIFO order decides arrival: x0..x4 first, the at
            # halves mid-stream (needed for the incremental folds, but not
            # before ~half the x tiles), tail x chunks last.
            x_sb = [xpool.tile([128, L], dtx, tag="x", name=f"xt{i}")
                    for i in range(PT)]
            half = PT * N // 2
            for pt in range(PT - 1):
                rings[pt % 2].dma_start(
                    x_sb[pt][:], xT[pt * 128:(pt + 1) * 128, :])
                if pt == 2:
                    rings[0].dma_start(at_sb[:, 0:half], atr[:, 0:half])
                elif pt == 3:
                    rings[1].dma_start(at_sb[:, half:], atr[:, half:])
            step = L // tail_split
            for j in range(tail_split):
                rings[(j + 1) % 2].dma_start(
                    x_sb[PT - 1][:, j * step:(j + 1) * step],
                    xT[(PT - 1) * 128:, j * step:(j + 1) * step],
                )

            c_ps = cps.tile([1, 1], f32, tag="cps")
            w8_acc = spool.tile([128, PT], f32, tag="w8acc")

            def to_mm_dtype(xs, scale=1.0):
                """MM operands must match at_sb's dtype; fold in any scale."""
                if dtx == f32 and scale == 1.0:
                    return xs
                xm = xsums.tile([128, 1], dtx, tag="xsmm")
                if scale != 1.0:
                    nc.vector.tensor_scalar_mul(xm[:], xs[:], scale)
                else:
                    nc.vector.tensor_copy(xm[:], xs[:])
                return xm

            def fold_ptile(pt, xs, scale=1.0):
                """Add Abar-block @ xsum_pt into w8_acc / c_ps."""
                xm = to_mm_dtype(xs, scale)
                wp = wps.tile([128, PT], f32, tag="wp")
                for nt in range(PT):
                    nc.tensor.matmul(
                        wp[:, nt:nt + 1],
                        at_sb[:, pt * N + nt * 128: pt * N + (nt + 1) * 128],
                        xm[:],
                        start=True, stop=True,
                    )
                nc.tensor.matmul(
                    c_ps[:], qv_sb[:, pt:pt + 1], xm[:],
                    start=(pt == 0), stop=(pt == PT - 1),
                )
                if pt == 0:
                    nc.vector.tensor_copy(w8_acc[:], wp[:])
                else:
                    nc.vector.tensor_add(w8_acc[:], w8_acc[:], wp[:])

            # Row-sum engine split: tensor_reduce is a 1x-mode DVE op
            # (~4.4us/tile), so alternate tiles onto ScalarE via
            # activation(Copy, accum_out=...) to halve the reduction span.
            act_scr = cpool.tile([128, L], dtx, tag="ascr")
            tree_scr = cpool.tile([128, 3 * L // 4], dtx, tag="tscr")

            def rowsum(tile_, lo, w, xs_out, eng):
                """Row-sum of tile_[:, lo:lo+w] on DVE or ACT.

                "vtree" does two bf16 pairwise-add levels first: tensor_tensor
                has a 2x_1P uop for packed bf16 while tensor_reduce is stuck
                at 1x, so this runs ~1.6x faster on DVE at a tiny precision
                cost (partials stay small; final 1/4-width reduce is fp32).
                """
                if eng == "act":
                    nc.scalar.activation(
                        act_scr[:, 0:w], tile_[:, lo:lo + w],
                        mybir.ActivationFunctionType.Copy,
                        bias=0.0, accum_out=xs_out,
                    )
                    return
                if eng == "vtree" and dtx != f32:
                    h, q = w // 2, w // 4
                    nc.vector.tensor_add(
                        tree_scr[:, 0:h],
                        tile_[:, lo:lo + h], tile_[:, lo + h:lo + w])
                    nc.vector.tensor_add(
                        tree_scr[:, h:h + q],
                        tree_scr[:, 0:q], tree_scr[:, q:h])
                    nc.vector.tensor_reduce(
                        xs_out, tree_scr[:, h:h + q],
                        axis=mybir.AxisListType.X, op=mybir.AluOpType.add,
                    )
                    return
                nc.vector.tensor_reduce(
                    xs_out, tile_[:, lo:lo + w], axis=mybir.AxisListType.X,
                    op=mybir.AluOpType.add,
                )

            ENG = ["act", "vtree", "act", "vtree", "act", "vtree", "act"]

            def rowsum_split(pt, xs_out, first_eng):
                """4-slice row-sum across both engines: cuts the post-DMA
                latency of a late-arriving tile from ~3.7us to ~1.4us."""
                nsl = 4
                w = L // nsl
                pr = xsums.tile([128, nsl], f32, tag="parts", name=f"pr{pt}")
                for j in range(nsl):
                    eng = ("act", "vtree")[(j + (first_eng == "vtree")) % 2]
                    rowsum(x_sb[pt], j * w, w, pr[:, j:j + 1], eng)
                nc.vector.tensor_reduce(
                    xs_out, pr[:], axis=mybir.AxisListType.X,
                    op=mybir.AluOpType.add,
                )

            # -- pass 1: reduce + incremental fold --
            # The last two full tiles land near the DMA tail; slice their
            # row-sums across both engines so no 3.7us unit gates fold7.
            # The HAM warmup matmuls are emitted BEFORE fold5 in the PE
            # stream: they trigger on the tail tile's first chunk (end of
            # DMA) and run while the PE would idle waiting for the late
            # xsum5/xsum6 — never on the fold6->fold7->pass2 path.
            for pt in range(PT - 1):
                if pt == PT - 3 and warmup_mms:
                    wscr = wrm.tile([1, LCH], f32, tag="warm")
                    for i in range(warmup_mms):
                        nc.tensor.matmul(
                            wscr[:], qv_sb[:, 0:1], x_sb[PT - 1][:, 0:LCH],
                            start=(i == 0), stop=(i == warmup_mms - 1),
                        )
                xs = xsums.tile([128, 1], f32, tag="xsum")
                rowsum_split(pt, xs[:], ENG[pt])
                fold_ptile(pt, xs)

            # tail tile: chunked reduce to shorten the critical path
            pt = PT - 1
            if tail_split > 1:
                step = L // tail_split
                parts = xsums.tile([128, tail_split], f32, tag="parts")
                for j in range(tail_split):
                    rowsum(x_sb[pt], j * step, step,
                           parts[:, j:j + 1], "act" if j % 2 == 0 else "vtree")
                xs = xsums.tile([128, 1], f32, tag="xsum")
                nc.vector.tensor_reduce(
                    xs[:], parts[:], axis=mybir.AxisListType.X,
                    op=mybir.AluOpType.add,
                )
            else:
                xs = xsums.tile([128, 1], f32, tag="xsum")
                rowsum(x_sb[pt], 0, L, xs[:], "vec")
            fold_ptile(pt, xs)

            # -- finalize w_eff / c --
            w_eff = spool.tile([128, PT], dtx, tag="weff")
            nc.vector.tensor_add(w_eff[:], w8_acc[:], u_sb[:])
            c_sb = spool.tile([1, 1], f32, tag="csb")
            nc.vector.tensor_add(c_sb[:], c_ps[:], c0_sb[:])

            # -- pass 2: out[l] = xT[:, l] . w_eff + c --
            # Per-chunk output DMAs overlap the remaining matmul groups;
            # only the last chunk's small store sits on the tail.
            out_sb = cpool.tile([1, L], f32, tag="osb")
            for lc in range(NLC):
                o_ps = ops.tile([1, LCH], f32, tag="ops")
                for nt in range(PT):
                    nc.tensor.matmul(
                        o_ps[:],
                        w_eff[:, nt:nt + 1],
                        x_sb[nt][:, lc * LCH:(lc + 1) * LCH],
                        start=(nt == 0), stop=(nt == PT - 1),
                    )
                nc.vector.tensor_scalar_add(
                    out_sb[0:1, lc * LCH:(lc + 1) * LCH], o_ps[:], c_sb[0:1, 0:1],
                )
                rings[lc % 2].dma_start(
                    out_d[0:1, lc * LCH:(lc + 1) * LCH],
                    out_sb[0:1, lc * LCH:(lc + 1) * LCH],
                )

    nc.compile()
    return nc


def _prep_host(inputs, x_dt_name="float32"):
    """Fold weights on host (f64 accumulate) and lay out per-core arrays."""
    Wq = np.asarray(inputs["Wq"], np.float64)
    bq = np.asarray(inputs["bq"], np.float64)
    Wk = np.asarray(inputs["Wk"], np.float64)
    bk = np.asarray(inputs["bk"], np.float64)
    Wfc = np.asarray(inputs["Wfc"], np.float64)
    bfc = np.asarray(inputs["bfc"], np.float64)

    s = np.repeat(Wfc[0], D_K) / np.sqrt(D_K)
    A = (Wk * s[:, None]).T @ Wq / L          # [n, p]
    u = Wk.T @ (s * bq)                       # [n]
    qv = Wq.T @ (s * bk) / L                  # [p]
    c0 = float((s * bk) @ bq + bfc[0])

    np_dtx = {"float32": np.float32, "bfloat16": None}[x_dt_name]
    if np_dtx is None:
        import ml_dtypes
        np_dtx = ml_dtypes.bfloat16

    at = np.ascontiguousarray(A.T)            # [p, n]
    atr = np.ascontiguousarray(
        at.reshape(PT, 128, N).transpose(1, 0, 2).reshape(128, PT * N)
    ).astype(np_dtx)
    qv8 = np.ascontiguousarray(qv.reshape(PT, 128).T).astype(np_dtx)
    u8 = np.ascontiguousarray(u.reshape(PT, 128).T).astype(np.float32)
    c0a = np.full((1, 1), c0, np.float32)

    x = np.asarray(inputs["x"])
    shared = {"atr": atr, "qv8": qv8, "u8": u8, "c0": c0a}
    in_maps = []
    for c in range(NCORES):
        m = dict(shared)
        m["xT"] = np.ascontiguousarray(x[c].T).astype(np_dtx, copy=False)
        in_maps.append(m)
    return in_maps


_X_DT = os.environ.get("KERNEL_X_DT", "bfloat16")
LAST_RESULTS = None


def kernel(**inputs) -> np.ndarray:
    global LAST_RESULTS
    _ensure_path()
    from concourse.bass_utils import run_bass_kernel_spmd

    nc = _build(_X_DT)
    in_maps = _prep_host(inputs, _X_DT)
    kw = {}
    if os.environ.get("KERNEL_TRACE"):
        kw["trace"] = True
    res = run_bass_kernel_spmd(nc, in_maps, list(range(NCORES)), **kw)
    LAST_RESULTS = res
    out = np.stack([res.results[c]["out"].reshape(L, 1) for c in range(NCORES)])
    return out.astype(np.float32)


if __name__ == "__main__":
    rng = np.random.default_rng(0)
    demo = {
        "x": rng.standard_normal((B, L, N), np.float32),
        "Wq": rng.standard_normal((N, N), np.float32) * 0.03,
        "bq": rng.standard_normal((N,), np.float32) * 0.03,
        "Wk": rng.standard_normal((N, N), np.float32) * 0.03,
        "bk": rng.standard_normal((N,), np.float32) * 0.03,
        "Wfc": rng.standard_normal((1, 16), np.float32) * 0.25,
        "bfc": rng.standard_normal((1,), np.float32) * 0.25,
    }
    o = kernel(**demo)
    print("out", o.shape, o.dtype, float(np.abs(o).max()))


# revision 45
# speedup vs baseline: 1.1255x; 1.1255x over previous
"""Trainium2 Bass kernel for nn_MultiHeadAttention_8684423872640.

Math: the reference collapses algebraically. With
  s[m]   = Wfc[0, m // 64] / sqrt(64)
  Abar   = (Wk * s[:,None]).T @ Wq / L          # [1024, 1024] weights-only
  u      = Wk.T @ (s * bq)                      # [1024]
  qv     = Wq.T @ (s * bk) / L                  # [1024]
  c0     = (s * bk) @ bq + bfc[0]
the output for batch b is
  xsum_b = sum_l x[b, l, :]                     # [1024]
  w_eff  = Abar @ xsum_b + u                    # [1024]
  c      = qv @ xsum_b + c0
  out[b, l, 0] = x[b, l, :] @ w_eff + c

Sharding: data-parallel over B — core c handles batch c. Each core:
  pass 1: DMA x[b].T tiles [128, 4096] to SBUF, VectorE row-sums -> xsum
          (incrementally per 128-feature tile), TensorE folds each xsum
          p-tile into w_eff/c via Abar-block matmuls as soon as it's ready
  pass 2: TensorE matvec out[l] = xT[:, l] . w_eff (w_eff stationary,
          x streams as moving operand), +c epilogue on VectorE, DMA out.
"""

import os
import sys
import functools
import numpy as np

B, L, N = 8, 4096, 1024
D_K = 64
NCORES = 8
PT = N // 128  # 8 feature tiles
LCH = 512      # pass-2 moving chunk (fp32 max)
NLC = L // LCH

_TRN_REPO = "/opt/trn_rl_repo"


def _ensure_path():
    if _TRN_REPO not in sys.path and os.path.isdir(_TRN_REPO):
        sys.path.insert(0, _TRN_REPO)


@functools.lru_cache(maxsize=2)
def _build(x_dt_name: str = "float32", tail_split: int = 4, warmup_mms: int = 10):
    """Build + compile the per-core Bass program. Returns the finalized nc."""
    _ensure_path()
    import concourse.bass as bass
    import concourse.tile as tile
    from concourse import bacc, mybir

    f32 = mybir.dt.float32
    dtx = getattr(mybir.dt, x_dt_name)

    nc = bacc.Bacc(
        "TRN2",
        target_bir_lowering=False,
        debug=False,
        enable_asserts=False,
        num_devices=NCORES,
    )

    xT = nc.dram_tensor("xT", [N, L], dtx, kind="ExternalInput").ap()
    atr = nc.dram_tensor("atr", [128, PT * N], dtx, kind="ExternalInput").ap()
    qv8 = nc.dram_tensor("qv8", [128, PT], dtx, kind="ExternalInput").ap()
    u8 = nc.dram_tensor("u8", [128, PT], f32, kind="ExternalInput").ap()
    c0 = nc.dram_tensor("c0", [1, 1], f32, kind="ExternalInput").ap()
    out_d = nc.dram_tensor("out", [1, L], f32, kind="ExternalOutput").ap()

    with tile.TileContext(nc) as tc:
        with (
            tc.tile_pool(name="xpool", bufs=PT) as xpool,
            tc.tile_pool(name="cpool", bufs=1) as cpool,
            tc.tile_pool(name="spool", bufs=2) as spool,
            tc.tile_pool(name="xsums", bufs=PT + 2) as xsums,
            tc.tile_pool(name="wps", bufs=2, space="PSUM") as wps,
            tc.tile_pool(name="cps", bufs=1, space="PSUM") as cps,
            tc.tile_pool(name="ops", bufs=3, space="PSUM") as ops,
            tc.tile_pool(name="wrm", bufs=1, space="PSUM") as wrm,
        ):
            # Three DMA queues: the two HWDGE rings (SP + ACT) carry x
            # tiles in alternation (one ring's ~2us completion receipt
            # hides under the other's data); weights ride SWDGE (gpsimd)
            # so they never delay an x tile.
            rings = [nc.sync, nc.scalar]

            # -- small constants up front (SWDGE; tiny) --
            at_sb = cpool.tile([128, PT * N], dtx, tag="at")
            qv_sb = cpool.tile([128, PT], dtx, tag="qv")
            nc.gpsimd.dma_start(qv_sb[:], qv8[:])
            u_sb = cpool.tile([128, PT], f32, tag="u")
            nc.gpsimd.dma_start(u_sb[:], u8[:])
            c0_sb = cpool.tile([1, 1], f32, tag="c0")
            nc.gpsimd.dma_start(c0_sb[:], c0[:])

            # -- x tiles interleaved with the two at halves --
            # Per-ring FIFO order decides arrival: x0..x4 first, the at
            # halves mid-stream (needed for the incremental folds, but not
            # before ~half the x tiles), tail x chunks last.
            x_sb = [xpool.tile([128, L], dtx, tag="x", name=f"xt{i}")
                    for i in range(PT)]
            half = PT * N // 2
            for pt in range(PT - 1):
                rings[pt % 2].dma_start(
                    x_sb[pt][:], xT[pt * 128:(pt + 1) * 128, :])
                if pt == 2:
                    rings[0].dma_start(at_sb[:, 0:half], atr[:, 0:half])
                elif pt == 3:
                    rings[1].dma_start(at_sb[:, half:], atr[:, half:])
            step = L // tail_split
            for j in range(tail_split):
                rings[(j + 1) % 2].dma_start(
                    x_sb[PT - 1][:, j * step:(j + 1) * step],
                    xT[(PT - 1) * 128:, j * step:(j + 1) * step],
                )

            c_ps = cps.tile([1, 1], f32, tag="cps")
            w8_acc = spool.tile([128, PT], f32, tag="w8acc")

            def to_mm_dtype(xs, scale=1.0):
                """MM operands must match at_sb's dtype; fold in any scale."""
                if dtx == f32 and scale == 1.0:
                    return xs
                xm = xsums.tile([128, 1], dtx, tag="xsmm")
                if scale != 1.0:
                    nc.vector.tensor_scalar_mul(xm[:], xs[:], scale)
                else:
                    nc.vector.tensor_copy(xm[:], xs[:])
                return xm

            def fold_ptile(pt, xs, scale=1.0):
                """Add Abar-block @ xsum_pt into w8_acc / c_ps."""
                xm = to_mm_dtype(xs, scale)
                wp = wps.tile([128, PT], f32, tag="wp")
                for nt in range(PT):
                    nc.tensor.matmul(
                        wp[:, nt:nt + 1],
                        at_sb[:, pt * N + nt * 128: pt * N + (nt + 1) * 128],
                        xm[:],
                        start=True, stop=True,
                    )
                nc.tensor.matmul(
                    c_ps[:], qv_sb[:, pt:pt + 1], xm[:],
                    start=(pt == 0), stop=(pt == PT - 1),
                )
                if pt == 0:
                    nc.vector.tensor_copy(w8_acc[:], wp[:])
                else:
                    nc.vector.tensor_add(w8_acc[:], w8_acc[:], wp[:])

            # Row-sum engine split: tensor_reduce is a 1x-mode DVE op
            # (~4.4us/tile), so alternate tiles onto ScalarE via
            # activation(Copy, accum_out=...) to halve the reduction span.
            act_scr = cpool.tile([128, L], dtx, tag="ascr")
            tree_scr = cpool.tile([128, 3 * L // 4], dtx, tag="tscr")

            def rowsum(tile_, lo, w, xs_out, eng):
                """Row-sum of tile_[:, lo:lo+w] on DVE or ACT.

                "vtree" does two bf16 pairwise-add levels first: tensor_tensor
                has a 2x_1P uop for packed bf16 while tensor_reduce is stuck
                at 1x, so this runs ~1.6x faster on DVE at a tiny precision
                cost (partials stay small; final 1/4-width reduce is fp32).
                """
                if eng == "act":
                    nc.scalar.activation(
                        act_scr[:, 0:w], tile_[:, lo:lo + w],
                        mybir.ActivationFunctionType.Copy,
                        bias=0.0, accum_out=xs_out,
                    )
                    return
                if eng == "vtree" and dtx != f32:
                    h, q = w // 2, w // 4
                    nc.vector.tensor_add(
                        tree_scr[:, 0:h],
                        tile_[:, lo:lo + h], tile_[:, lo + h:lo + w])
                    nc.vector.tensor_add(
                        tree_scr[:, h:h + q],
                        tree_scr[:, 0:q], tree_scr[:, q:h])
                    nc.vector.tensor_reduce(
                        xs_out, tree_scr[:, h:h + q],
                        axis=mybir.AxisListType.X, op=mybir.AluOpType.add,
                    )
                    return
                nc.vector.tensor_reduce(
                    xs_out, tile_[:, lo:lo + w], axis=mybir.AxisListType.X,
                    op=mybir.AluOpType.add,
                )

            ENG = ["act", "vtree", "act", "vtree", "act", "vtree", "act"]

            def rowsum_split(pt, xs_out, first_eng):
                """4-slice row-sum across both engines: cuts the post-DMA
                latency of a late-arriving tile from ~3.7us to ~1.4us."""
                nsl = 4
                w = L // nsl
                pr = xsums.tile([128, nsl], f32, tag="parts", name=f"pr{pt}")
                for j in range(nsl):
                    eng = ("act", "vtree")[(j + (first_eng == "vtree")) % 2]
                    rowsum(x_sb[pt], j * w, w, pr[:, j:j + 1], eng)
                nc.vector.tensor_reduce(
                    xs_out, pr[:], axis=mybir.AxisListType.X,
                    op=mybir.AluOpType.add,
                )

            # -- pass 1: reduce + incremental fold --
            # The last two full tiles land near the DMA tail; slice their
            # row-sums across both engines so no 3.7us unit gates fold7.
            # The HAM warmup matmuls are emitted BEFORE fold5 in the PE
            # stream: they trigger on the tail tile's first chunk (end of
            # DMA) and run while the PE would idle waiting for the late
            # xsum5/xsum6 — never on the fold6->fold7->pass2 path.
            for pt in range(PT - 1):
                if pt == PT - 3 and warmup_mms:
                    wscr = wrm.tile([1, LCH], f32, tag="warm")
                    for i in range(warmup_mms):
                        nc.tensor.matmul(
                            wscr[:], qv_sb[:, 0:1], x_sb[PT - 1][:, 0:LCH],
                            start=(i == 0), stop=(i == warmup_mms - 1),
                        )
                xs = xsums.tile([128, 1], f32, tag="xsum")
                if pt >= PT - 3:
                    rowsum_split(pt, xs[:], ENG[pt])
                else:
                    rowsum(x_sb[pt], 0, L, xs[:], ENG[pt])
                fold_ptile(pt, xs)

            # tail tile: chunked reduce to shorten the critical path
            pt = PT - 1
            if tail_split > 1:
                step = L // tail_split
                parts = xsums.tile([128, tail_split], f32, tag="parts")
                for j in range(tail_split):
                    rowsum(x_sb[pt], j * step, step,
                           parts[:, j:j + 1], "act" if j % 2 == 0 else "vtree")
                xs = xsums.tile([128, 1], f32, tag="xsum")
                nc.vector.tensor_reduce(
                    xs[:], parts[:], axis=mybir.AxisListType.X,
                    op=mybir.AluOpType.add,
                )
            else:
                xs = xsums.tile([128, 1], f32, tag="xsum")
                rowsum(x_sb[pt], 0, L, xs[:], "vec")
            fold_ptile(pt, xs)

            # -- finalize w_eff / c --
            w_eff = spool.tile([128, PT], dtx, tag="weff")
            nc.vector.tensor_add(w_eff[:], w8_acc[:], u_sb[:])
            c_sb = spool.tile([1, 1], f32, tag="csb")
            nc.vector.tensor_add(c_sb[:], c_ps[:], c0_sb[:])

            # -- pass 2: out[l] = xT[:, l] . w_eff + c --
            # Per-chunk output DMAs overlap the remaining matmul groups;
            # only the last chunk's small store sits on the tail.
            out_sb = cpool.tile([1, L], f32, tag="osb")
            for lc in range(NLC):
                o_ps = ops.tile([1, LCH], f32, tag="ops")
                for nt in range(PT):
                    nc.tensor.matmul(
                        o_ps[:],
                        w_eff[:, nt:nt + 1],
                        x_sb[nt][:, lc * LCH:(lc + 1) * LCH],
                        start=(nt == 0), stop=(nt == PT - 1),
                    )
                nc.vector.tensor_scalar_add(
                    out_sb[0:1, lc * LCH:(lc + 1) * LCH], o_ps[:], c_sb[0:1, 0:1],
                )
                rings[lc % 2].dma_start(
                    out_d[0:1, lc * LCH:(lc + 1) * LCH],
                    out_sb[0:1, lc * LCH:(lc + 1) * LCH],
                )

    nc.compile()
    return nc


def _prep_host(inputs, x_dt_name="float32"):
    """Fold weights on host (f64 accumulate) and lay out per-core arrays."""
    Wq = np.asarray(inputs["Wq"], np.float64)
    bq = np.asarray(inputs["bq"], np.float64)
    Wk = np.asarray(inputs["Wk"], np.float64)
    bk = np.asarray(inputs["bk"], np.float64)
    Wfc = np.asarray(inputs["Wfc"], np.float64)
    bfc = np.asarray(inputs["bfc"], np.float64)

    s = np.repeat(Wfc[0], D_K) / np.sqrt(D_K)
    A = (Wk * s[:, None]).T @ Wq / L          # [n, p]
    u = Wk.T @ (s * bq)                       # [n]
    qv = Wq.T @ (s * bk) / L                  # [p]
    c0 = float((s * bk) @ bq + bfc[0])

    np_dtx = {"float32": np.float32, "bfloat16": None}[x_dt_name]
    if np_dtx is None:
        import ml_dtypes
        np_dtx = ml_dtypes.bfloat16

    at = np.ascontiguousarray(A.T)            # [p, n]
    atr = np.ascontiguousarray(
        at.reshape(PT, 128, N).transpose(1, 0, 2).reshape(128, PT * N)
    ).astype(np_dtx)
    qv8 = np.ascontiguousarray(qv.reshape(PT, 128).T).astype(np_dtx)
    u8 = np.ascontiguousarray(u.reshape(PT, 128).T).astype(np.float32)
    c0a = np.full((1, 1), c0, np.float32)

    x = np.asarray(inputs["x"])
    shared = {"atr": atr, "qv8": qv8, "u8": u8, "c0": c0a}
    in_maps = []
    for c in range(NCORES):
        m = dict(shared)
        m["xT"] = np.ascontiguousarray(x[c].T).astype(np_dtx, copy=False)
        in_maps.append(m)
    return in_maps


_X_DT = os.environ.get("KERNEL_X_DT", "bfloat16")
LAST_RESULTS = None


def kernel(**inputs) -> np.ndarray:
    global LAST_RESULTS
    _ensure_path()
    from concourse.bass_utils import run_bass_kernel_spmd

    nc = _build(_X_DT)
    in_maps = _prep_host(inputs, _X_DT)
    kw = {}
    if os.environ.get("KERNEL_TRACE"):
        kw["trace"] = True
    res = run_bass_kernel_spmd(nc, in_maps, list(range(NCORES)), **kw)
    LAST_RESULTS = res
    out = np.stack([res.results[c]["out"].reshape(L, 1) for c in range(NCORES)])
    return out.astype(np.float32)


if __name__ == "__main__":
    rng = np.random.default_rng(0)
    demo = {
        "x": rng.standard_normal((B, L, N), np.float32),
        "Wq": rng.standard_normal((N, N), np.float32) * 0.03,
        "bq": rng.standard_normal((N,), np.float32) * 0.03,
        "Wk": rng.standard_normal((N, N), np.float32) * 0.03,
        "bk": rng.standard_normal((N,), np.float32) * 0.03,
        "Wfc": rng.standard_normal((1, 16), np.float32) * 0.25,
        "bfc": rng.standard_normal((1,), np.float32) * 0.25,
    }
    o = kernel(**demo)
    print("out", o.shape, o.dtype, float(np.abs(o).max()))
